# revision 56
# baseline (speedup 1.0000x reference)
"""Trainium2 Bass kernel for the CIN-style bilinear layer:

    out[b, o] = sum_{f,p} x0[b,f] * x[b,p] * W[o,f,p] + bias[o]

i.e. a batch of bilinear forms (outer-product + full-window Conv1d), shapes
B=4096, FIELD=128, H_PREV=128, H_NEXT=256, returned as [B, 256, 1] fp32.

Strategy (data-parallel over 8 NeuronCores, batch sharded 512/core):

The contraction index k=(f,p) has size 16384.  Tile it as 128 K-tiles of 128,
where each K-tile kt=(fb,pb) covers a 16x8 block of (f,p): f = fb*16+fi,
p = pb*8+pi, and the within-tile index is k = fi*8+pi.  The outer-product
operand for one K-tile is then

    A[k, kt, b] = x0T[f(k), b] * xT[p(k), b]

which is an elementwise product of two *small-factor replicated* tensors:
rep8_x0t[k, fb, b] = x0T[fb*16 + k//8, b]   (8x partition replication, 1MB)
rep16_xt[k, pb, b] = xT[pb*8 + k%8, b]      (16x partition replication, 2MB)

Both replications are done on the host (pure data marshalling), so the device
builds A with plain vector tensor_mul ops (fp16, 2x DVE mode, no partition
broadcast needed, no transposes).  The tensor engine accumulates the output
transposed, out_t[o, b] (2 PSUM banks, one per 128-wide o-half), with the
host-pre-permuted W slice [k, o] as the stationary operand and A[k, b] as the
N=512 moving operand: 256 accumulating matmuls per core.  The fp32 bias is
fused into the PSUM->SBUF eviction as a per-partition tensor_scalar_add.

Mixed precision (default f8n2+f8h): 1.5 of the 8 fb blocks (24 of 128
K-tiles) run in fp8e4m3 DoubleRow, contracting 2 K-tiles per matmul at 0.5
cycles/row.  Their A slices are ACT-cast fp16->fp8 (the ACT engine is
otherwise idle), and their W slices are host-quantized at x2^10 scale
(uniform(-1/128,1/128) sits in e4m3's subnormal range unscaled), accumulated
in 2 separate PSUM banks and rescaled by 2^-10 during the eviction merge.
fp16 blocks keep fp32-exact accumulation; measured max-rel-error vs the fp32
reference is 1.83e-2 (threshold 2e-2; pure-fp16 variant "w_stat_512" is
4.3e-4 at ~82us vs ~75us mixed).
"""

import numpy as np

import concourse.bacc as bacc
import concourse.mybir as mybir
import concourse.tile as tile
from concourse.bass_utils import run_bass_kernel_spmd

B, F, P, O = 4096, 128, 128, 256
NCORES = 8
BC = B // NCORES  # 512 batch elements per core
FI, PI = 16, 8  # within-K-tile block: k = fi*PI + pi
FB, PB = F // FI, P // PI  # 8 f-blocks, 16 p-blocks; kt = fb*PB + pb
BT = BC // 128  # 4 batch tiles of 128 per core

_F16 = mybir.dt.float16
_F32 = mybir.dt.float32
_F8 = mybir.dt.float8e4
_NP8 = mybir.dt.np(_F8)

# which fb blocks run in fp8 DoubleRow for a given nfp8 (fb0 stays fp16 —
# it's the latency-critical prologue block); spread so the slow (DVE+ACT)
# fp8 A-builds interleave with fast fp16 ones
FP8_FBS_MAP = {
    0: (),
    1: (4,),
    2: (3, 6),
    3: (2, 4, 6),
    4: (1, 3, 5, 7),
    5: (1, 2, 4, 5, 7),
    6: (1, 2, 3, 5, 6, 7),
}

_NC_CACHE = {}


VARIANT = "w_stat_512+f8n2+f8h+stag+bufs3+ttc8+oq"


def _build_nc(repeat=1, loop_n=0, variant=None):
    """Build + compile the (SPMD, per-core) bass program once.

    repeat>1 re-emits the whole kernel body back-to-back (sharing tile pools,
    so SBUF stays bounded); loop_n>0 additionally wraps the body in a
    hardware For_i loop.  Both are used only by the benchmark harness to
    measure steady-state per-iteration device time via slopes (the axon
    dispatch overhead per call is ~80ms, so single-shot wall time is
    useless).
    """
    if variant is None:
        variant = VARIANT
    key = (repeat, loop_n, variant)
    if key in _NC_CACHE:
        return _NC_CACHE[key]
    n_warm = 0
    opts = variant.split("+")
    variant = opts[0]
    nbufs = 2
    tt_chunks = 2
    hosta = False
    use_q2 = False
    stag = False
    fine = False
    bf16 = False
    horder = False
    ps4 = False
    etail = False
    nfp8 = 0
    f8h = False
    ilv = False
    dro = False
    aev = False
    oq = False
    for o in opts[1:]:
        if o.startswith("warm"):
            n_warm = int(o[4:])
        elif o.startswith("bufs"):
            nbufs = int(o[4:])
        elif o.startswith("ttc"):
            tt_chunks = int(o[3:])
        elif o == "hosta":
            hosta = True
        elif o == "q2":
            use_q2 = True
        elif o == "stag":
            stag = True
        elif o == "fine":
            fine = True
        elif o == "bf16":
            bf16 = True
        elif o == "horder":
            horder = True
        elif o == "etail":
            etail = True
        elif o.startswith("f8n"):
            nfp8 = int(o[3:])
        elif o == "f8h":
            f8h = True
        elif o == "ilv":
            ilv = True
        elif o == "dro":
            dro = True
        elif o == "aev":
            aev = True
        elif o == "oq":
            oq = True
        elif o == "ps4":
            ps4 = True

    nc = bacc.Bacc(
        "TRN2", target_bir_lowering=False, debug=False, num_devices=NCORES
    )

    dt16 = mybir.dt.bfloat16 if bf16 else _F16

    if variant in ("pe_only", "pe8_only", "dve_only", "dma_only", "dma_split"):
        return _build_probe_nc(nc, variant, repeat, loop_n, dt16, key, nfp8)

    fp8_fbs = FP8_FBS_MAP[nfp8]
    if nfp8:
        assert variant == "w_stat_512" and not (etail or horder or ps4 or hosta)
    assert 0 not in fp8_fbs
    assert not (f8h and nfp8 < 2), "half block needs a full fp8 block first"
    half_fb = fp8_fbs[-1] if (f8h and nfp8) else None
    # blocks with any fp16 weights (the half block appears in both tensors:
    # its pb 0-7 run fp16, pb 8-15 run fp8 DoubleRow)
    fbs16 = [f for f in range(FB) if f not in fp8_fbs or f == half_fb]
    n16 = len(fbs16)
    idx16 = {f: i for i, f in enumerate(fbs16)}
    idx8 = {f: i for i, f in enumerate(fp8_fbs)}
    # ilv: each full-fp8 block pairs with the preceding pure-fp16 block; its
    # DR matmuls interleave 1-per-pb into that block's 2-per-pb stream so the
    # 256-col DR LDWEIGHTS can hide under the longer fp16 matmuls
    ilv_pairs = {}
    if ilv and nfp8:
        for f in fp8_fbs:
            if f == half_fb or f - 1 < 0:
                continue
            if f - 1 in fbs16 and f - 1 != half_fb and f - 1 not in fp8_fbs:
                ilv_pairs[f - 1] = f

    rep8 = nc.declare_dram_parameter("rep8_x0t", [128, FB, BC], dt16, isOutput=False)
    rep16 = nc.declare_dram_parameter("rep16_xt", [128, PB, BC], dt16, isOutput=False)
    w_re = nc.declare_dram_parameter("w_re", [n16, 128, PB, O], dt16, isOutput=False)
    if nfp8:
        w8_re = nc.declare_dram_parameter(
            "w8_re", [nfp8, 128, PB, O], _F8, isOutput=False
        )
    a_fb0 = nc.declare_dram_parameter("a_fb0", [128, PB, BC], dt16, isOutput=False)
    bias = nc.declare_dram_parameter("bias_col", [O, 1], _F32, isOutput=False)
    # output is stored transposed: out_t[o, b] (host transposes back)
    out = nc.declare_dram_parameter("out_t", [O, BC], _F32, isOutput=True)

    with tile.TileContext(nc) as tc:
        import contextlib

        loop_ctx = (
            tc.For_i(
                0,
                loop_n,
                1,
                hint_engines=(
                    mybir.EngineType.PE,
                    mybir.EngineType.DVE,
                    mybir.EngineType.SP,
                    mybir.EngineType.Activation,
                ),
                staggered_reset=stag,
            )
            if loop_n
            else contextlib.nullcontext()
        )
        with (
            loop_ctx,
            tc.tile_pool(name="inp", bufs=2) as inp,
            tc.tile_pool(name="wp", bufs=nbufs) as wp,
            tc.tile_pool(name="wp8", bufs=nbufs) as wp8,
            tc.tile_pool(name="ap", bufs=nbufs) as ap_pool,
            tc.tile_pool(name="ap8", bufs=nbufs) as ap8_pool,
            tc.tile_pool(name="op", bufs=2) as op,
            tc.tile_pool(name="ps", bufs=1, space="PSUM") as psp,
        ):
            for _rep in range(repeat):
                # prologue is latency-critical: the first matmuls need only
                # rep8[:, 0], rep16[:, 0:4], and W[0][:, 0:4] — load those
                # first in small chunks (the sync HWDGE queue is FIFO, so
                # emission order = arrival order), stream the rest behind
                rep8_sb = inp.tile([128, FB, BC], dt16, tag="rep8")
                rep16_sb = inp.tile([128, PB, BC], dt16, tag="rep16")
                w_sb_first = wp.tile([128, PB, O], dt16, tag="w", name="w_sb")
                NQ = PB // 4
                if hosta:
                    # fb=0's A comes precomputed from the host: the critical
                    # chain is just two small DMAs (A chunk + W chunk); the
                    # rep8/rep16 loads (needed from fb=1 on) ride the second
                    # HWDGE queue (scalar engine) in parallel
                    a_sb_first = ap_pool.tile(
                        [128, PB, BC], dt16, tag="a", name="a_sb"
                    )
                    pbs = (0, 2, 4, 8, 12, 16) if fine else (0, 4, 8, 12, 16)
                    for lo, hi in zip(pbs[:-1], pbs[1:]):
                        sl = slice(lo, hi)
                        nc.sync.dma_start(a_sb_first[:, sl, :], a_fb0[:, sl, :])
                        nc.sync.dma_start(w_sb_first[:, sl, :], w_re[0][:, sl, :])
                    nc.scalar.dma_start(rep8_sb[:], rep8[:])
                    for q in range(4):
                        sl = slice(q * NQ, (q + 1) * NQ)
                        nc.scalar.dma_start(rep16_sb[:, sl, :], rep16[:, sl, :])
                else:
                    nc.sync.dma_start(rep8_sb[:, 0:1, :], rep8[:, 0:1, :])
                    # chunk boundaries for the fb=0 critical loads: finer at
                    # the head so the first matmul starts as early as possible
                    pbs = (0, 2, 4, 8, 12, 16) if fine else (0, 4, 8, 12, 16)
                    for lo, hi in zip(pbs[:-1], pbs[1:]):
                        sl = slice(lo, hi)
                        nc.sync.dma_start(rep16_sb[:, sl, :], rep16[:, sl, :])
                        nc.sync.dma_start(w_sb_first[:, sl, :], w_re[0][:, sl, :])
                    if use_q2:
                        nc.scalar.dma_start(rep8_sb[:, 1:FB, :], rep8[:, 1:FB, :])
                    else:
                        nc.sync.dma_start(rep8_sb[:, 1:FB, :], rep8[:, 1:FB, :])
                bias_sb = inp.tile([128, O // 128], _F32, tag="bias")
                for h in range(O // 128):
                    nc.sync.dma_start(
                        bias_sb[:, h : h + 1], bias[h * 128 : (h + 1) * 128, :]
                    )

                # PE warmup experiment (measured net-negative, default off):
                # dummy matmuls during the prologue to pre-release the HAM
                # clock gate — the cold-rate dummies delay the real stream
                # more than the warm clock saves.
                if n_warm:
                    warm_sb = inp.tile([1, BC], dt16, tag="warm")
                    nc.vector.memset(warm_sb[:], 0.0)
                    warm_ps = psp.tile(
                        [64, BC], _F32, tag="warmps", name="warm_ps"
                    )
                    for _wi in range(n_warm):
                        nc.tensor.matmul(
                            warm_ps[:],
                            warm_sb[:, 0:64],
                            warm_sb[:],
                            start=True,
                            stop=True,
                        )

                if nfp8:
                    # fp8 blocks accumulate separately: their W is scaled by
                    # 2^10 on the host (else uniform(-1/128,1/128) lands in
                    # e4m3's subnormal range, ~7x the mantissa error) and the
                    # 2^-10 rescale happens at eviction
                    psum8_tiles = [
                        psp.tile([128, BC], _F32, tag=f"acc8{h}", name=f"acc8{h}")
                        for h in range(O // 128)
                    ]
                if variant == "a_stat":
                    psum_bt_tiles = [
                        psp.tile([128, O], _F32, tag=f"bacc{bt}", name=f"bacc{bt}")
                        for bt in range(BT)
                    ]
                elif ps4:
                    # 4-bank rotation: even/odd pb sub-chains per o-half
                    psum_quads = [
                        [
                            psp.tile(
                                [128, BC], _F32, tag=f"q{h}{e}", name=f"q{h}{e}"
                            )
                            for e in range(2)
                        ]
                        for h in range(O // 128)
                    ]
                else:
                    # out.T[o, b] accumulators: one full PSUM bank per o-half
                    psum_tiles = [
                        psp.tile([128, BC], _F32, tag=f"acc{h}", name=f"acc{h}")
                        for h in range(O // 128)
                    ]

                def emit_fp8_loads(f8, half=False):
                    """DMA w8, build fp16 A on DVE (2x mode), ACT-cast to
                    fp8e4. half=True covers only pb 8..15 (the half block's
                    fp8 part); the caller builds the fp16 A itself then."""
                    pbn = PB // 2 if half else PB
                    off = PB // 2 if half else 0
                    w8_sb = wp8.tile([128, pbn, O], _F8, tag="w8", name="w8_sb")
                    nc.sync.dma_start(w8_sb[:], w8_re[idx8[f8]][:, off:, :])
                    a8_sb = ap8_pool.tile(
                        [128, pbn, BC], _F8, tag="a8", name="a8_sb"
                    )
                    if half:
                        return w8_sb, a8_sb
                    a16p = ap_pool.tile([128, PB, BC], dt16, tag="a", name="a_sb")
                    bounds = tuple(
                        i * (PB // tt_chunks) for i in range(tt_chunks + 1)
                    )
                    for lo, hi in zip(bounds[:-1], bounds[1:]):
                        sl = slice(lo, hi)
                        nc.vector.tensor_mul(
                            a16p[:, sl, :],
                            rep8_sb[:, f8 : f8 + 1, :].broadcast_to(
                                (128, hi - lo, BC)
                            ),
                            rep16_sb[:, sl, :],
                        )
                        nc.scalar.copy(a8_sb[:, sl, :], a16p[:, sl, :])
                    return w8_sb, a8_sb

                def emit_dr(p8, w8_sb, a8_sb, jp, hh, njp, off=0):
                    """One DoubleRow matmul; jp indexes the local a8/w8 pair,
                    off is the global pb offset (8 for the half block).
                    start/stop are per-PSUM-bank: they fire at the first/last
                    jp of the first/last fp8 block for BOTH h banks."""
                    st8 = p8 == fp8_fbs[0] and jp == 0 and off == 0
                    sp8 = p8 == fp8_fbs[-1] and jp == njp - 1
                    nc.tensor.matmul(
                        psum8_tiles[hh][:],
                        w8_sb[:, 2 * jp : 2 * jp + 2, hh * 128 : (hh + 1) * 128],
                        a8_sb[:, 2 * jp : 2 * jp + 2, :],
                        start=st8,
                        stop=sp8,
                        perf_mode=mybir.MatmulPerfMode.DoubleRow,
                    )

                handled_by_pair = set(ilv_pairs.values())
                for fb in range(FB):
                    if fb in handled_by_pair:
                        continue
                    if fb in fp8_fbs and fb != half_fb:
                        # standalone fp8 DoubleRow block
                        w8_sb, a8_sb = emit_fp8_loads(fb)
                        if dro:
                            # bank-grouped: all jp for h=0, then h=1
                            for hh in range(O // 128):
                                for jp in range(PB // 2):
                                    emit_dr(fb, w8_sb, a8_sb, jp, hh, PB // 2)
                        else:
                            for jp in range(PB // 2):
                                for hh in range(O // 128):
                                    emit_dr(fb, w8_sb, a8_sb, jp, hh, PB // 2)
                        continue
                    if fb == 0:
                        w_sb = w_sb_first
                    elif fb == half_fb:
                        # half block: fp16 weights for pb 0..7 only
                        w_sb = wp.tile([128, PB // 2, O], dt16, tag="wh", name="w_sbh")
                        nc.sync.dma_start(
                            w_sb[:], w_re[idx16[fb]][:, : PB // 2, :]
                        )
                    else:
                        w_sb = wp.tile([128, PB, O], dt16, tag="w", name="w_sb")
                        nc.sync.dma_start(w_sb[:], w_re[idx16[fb]])

                    if fb == half_fb:
                        w8h_sb, a8h_sb = emit_fp8_loads(fb, half=True)

                    if fb == 0 and hosta:
                        a_sb = a_sb_first
                    else:
                        a_sb = ap_pool.tile(
                            [128, PB, BC], dt16, tag="a", name="a_sb"
                        )
                        if fb == 0:
                            bounds = (0, 2, 4, 8, 12, 16) if fine else (0, 4, 8, 12, 16)
                        else:
                            bounds = tuple(
                                i * (PB // tt_chunks)
                                for i in range(tt_chunks + 1)
                            )
                        for lo, hi in zip(bounds[:-1], bounds[1:]):
                            sl = slice(lo, hi)
                            nc.vector.tensor_mul(
                                a_sb[:, sl, :],
                                rep8_sb[:, fb : fb + 1, :].broadcast_to(
                                    (128, hi - lo, BC)
                                ),
                                rep16_sb[:, sl, :],
                            )
                            if fb == half_fb and hi > PB // 2:
                                cs = max(lo, PB // 2)
                                nc.scalar.copy(
                                    a8h_sb[:, cs - PB // 2 : hi - PB // 2, :],
                                    a_sb[:, cs:hi, :],
                                )

                    first = fb == 0
                    # last block carrying fp16 matmuls — closes the fp16 group
                    last = fb == max(fbs16)
                    if fb == half_fb:
                        # half block: pb 0..7 fp16, pb 8..15 as 4 DR pairs
                        # interleaved (1 DR per pb slot) so their LDWs hide
                        dr_list = [
                            (jp, hh)
                            for jp in range(PB // 4)
                            for hh in range(O // 128)
                        ]
                        for pb in range(PB // 2):
                            sp = last and pb == PB // 2 - 1
                            for h in range(O // 128):
                                nc.tensor.matmul(
                                    psum_tiles[h][:],
                                    w_sb[:, pb, h * 128 : (h + 1) * 128],
                                    a_sb[:, pb, :],
                                    start=(first and pb == 0),
                                    stop=sp,
                                )
                            jp, hh = dr_list[pb]
                            emit_dr(
                                fb, w8h_sb, a8h_sb, jp, hh, PB // 4, off=PB // 2
                            )
                        continue
                    if fb in ilv_pairs:
                        # fp16 block with an interleaved fp8 partner: 2 fp16
                        # matmuls then 1 DR per pb slot
                        p8 = ilv_pairs[fb]
                        w8p_sb, a8p_sb = emit_fp8_loads(p8)
                        dr_list = [
                            (jp, hh)
                            for jp in range(PB // 2)
                            for hh in range(O // 128)
                        ]
                        for pb in range(PB):
                            st = first and pb == 0
                            sp = last and pb == PB - 1
                            for h in range(O // 128):
                                nc.tensor.matmul(
                                    psum_tiles[h][:],
                                    w_sb[:, pb, h * 128 : (h + 1) * 128],
                                    a_sb[:, pb, :],
                                    start=st,
                                    stop=sp,
                                )
                            jp, hh = dr_list[pb]
                            emit_dr(p8, w8p_sb, a8p_sb, jp, hh, PB // 2)
                        continue
                    if variant == "w_stat_512" and etail and last:
                        # epilogue overlap: finish bank h's accumulation
                        # first, evict it while the other bank's matmuls run
                        for h in range(O // 128):
                            for pb in range(PB):
                                nc.tensor.matmul(
                                    psum_tiles[h][:],
                                    w_sb[:, pb, h * 128 : (h + 1) * 128],
                                    a_sb[:, pb, :],
                                    start=False,
                                    stop=(pb == PB - 1),
                                )
                            out_sb = op.tile(
                                [128, BC], _F32, tag="out", name="out_sb"
                            )
                            nc.vector.tensor_scalar_add(
                                out_sb[:],
                                psum_tiles[h][:],
                                bias_sb[:, h : h + 1],
                            )
                            nc.sync.dma_start(
                                out[h * 128 : (h + 1) * 128, :], out_sb[:]
                            )
                        continue
                    if variant == "w_stat_512" and horder:
                        # same-bank matmuls grouped: all pb for h=0, then h=1
                        for h in range(O // 128):
                            for pb in range(PB):
                                nc.tensor.matmul(
                                    psum_tiles[h][:],
                                    w_sb[:, pb, h * 128 : (h + 1) * 128],
                                    a_sb[:, pb, :],
                                    start=(first and pb == 0),
                                    stop=(last and pb == PB - 1),
                                )
                        continue
                    if variant == "w_stat_512" and ps4:
                        for pb in range(PB):
                            e = pb % 2
                            for h in range(O // 128):
                                nc.tensor.matmul(
                                    psum_quads[h][e][:],
                                    w_sb[:, pb, h * 128 : (h + 1) * 128],
                                    a_sb[:, pb, :],
                                    start=(first and pb < 2),
                                    stop=(last and pb >= PB - 2),
                                )
                        continue
                    for pb in range(PB):
                        st = first and pb == 0
                        sp = last and pb == PB - 1
                        if variant == "w_stat_512":
                            # W stationary, full-batch moving (N=512)
                            for h in range(O // 128):
                                nc.tensor.matmul(
                                    psum_tiles[h][:],
                                    w_sb[:, pb, h * 128 : (h + 1) * 128],
                                    a_sb[:, pb, :],
                                    start=st,
                                    stop=sp,
                                )
                        elif variant == "w_stat_256":
                            # W stationary, two N=256 moving halves share
                            # one weight load
                            for h in range(O // 128):
                                for bh in range(2):
                                    nc.tensor.matmul(
                                        psum_tiles[h][
                                            :, bh * 256 : (bh + 1) * 256
                                        ],
                                        w_sb[:, pb, h * 128 : (h + 1) * 128],
                                        a_sb[:, pb, bh * 256 : (bh + 1) * 256],
                                        start=st,
                                        stop=sp,
                                        skip_group_check=True,
                                    )
                        elif variant == "a_stat":
                            # control: A-slices stationary, W moving,
                            # accumulating out[b, o] in 4 b-tile psums.
                            # BENCH-ONLY: no bias, output written scrambled.
                            for bt in range(BT):
                                nc.tensor.matmul(
                                    psum_bt_tiles[bt][:],
                                    a_sb[:, pb, bt * 128 : (bt + 1) * 128],
                                    w_sb[:, pb, :],
                                    start=st,
                                    stop=sp,
                                )
                        else:
                            raise ValueError(variant)

                if variant == "a_stat":
                    flat = out[:].rearrange("o b -> (o b)")
                    for bt in range(BT):
                        out_sb = op.tile([128, O], _F32, tag="out", name="out_sb")
                        nc.scalar.copy(out_sb[:], psum_bt_tiles[bt][:])
                        nc.sync.dma_start(
                            flat[bt * 128 * O : (bt + 1) * 128 * O].rearrange(
                                "(p f) -> p f", p=128
                            ),
                            out_sb[:],
                        )
                elif ps4:
                    for h in range(O // 128):
                        out_sb = op.tile([128, BC], _F32, tag="out", name="out_sb")
                        # merge the even/odd partial sums + bias with two ops
                        # (each reads at most one PSUM operand)
                        nc.vector.tensor_scalar_add(
                            out_sb[:], psum_quads[h][0][:], bias_sb[:, h : h + 1]
                        )
                        nc.vector.tensor_add(
                            out_sb[:], out_sb[:], psum_quads[h][1][:]
                        )
                        nc.sync.dma_start(
                            out[h * 128 : (h + 1) * 128, :], out_sb[:]
                        )
                elif not etail:
                    for h in range(O // 128):
                        out_sb = op.tile([128, BC], _F32, tag="out", name="out_sb")
                        if nfp8:
                            # out = (psum8 * 2^-10 + bias) + psum16
                            if aev:
                                # first op on the idle ACT engine so the two
                                # eviction stages pipeline across engines
                                nc.scalar.activation(
                                    out_sb[:],
                                    psum8_tiles[h][:],
                                    mybir.ActivationFunctionType.Identity,
                                    bias=bias_sb[:, h : h + 1],
                                    scale=1.0 / 1024.0,
                                )
                            else:
                                nc.vector.tensor_scalar(
                                    out_sb[:],
                                    psum8_tiles[h][:],
                                    1.0 / 1024.0,
                                    bias_sb[:, h : h + 1],
                                    mybir.AluOpType.mult,
                                    mybir.AluOpType.add,
                                )
                            nc.vector.tensor_add(
                                out_sb[:], out_sb[:], psum_tiles[h][:]
                            )
                        else:
                            # eviction fused with the (exact, fp32) bias add:
                            # out_t[o, b] = psum[o, b] + bias[o]
                            nc.vector.tensor_scalar_add(
                                out_sb[:], psum_tiles[h][:], bias_sb[:, h : h + 1]
                            )
                        (nc.scalar if oq else nc.sync).dma_start(
                            out[h * 128 : (h + 1) * 128, :], out_sb[:]
                        )

    nc.compile()
    _NC_CACHE[key] = nc
    return nc


def _build_probe_nc(nc, variant, repeat, loop_n, dt16, key, nfp8=0):
    """BENCH-ONLY engine-isolation probes (never used by kernel()):

    pe_only   — the full 256-matmul stream reading fixed (memset) SBUF tiles;
                no DMA / DVE in the loop body. Isolates PE stream time.
    dve_only  — the 16 a_sb tensor_mul builds from fixed tiles; no matmuls.
                Isolates DVE stream time.
    dma_only  — all input DMAs on the sync queue; no compute.
    dma_split — same but W rides the scalar queue, rest on sync.
    """
    import contextlib

    rep8 = nc.declare_dram_parameter("rep8_x0t", [128, FB, BC], dt16, isOutput=False)
    rep16 = nc.declare_dram_parameter("rep16_xt", [128, PB, BC], dt16, isOutput=False)
    w_re = nc.declare_dram_parameter("w_re", [FB, 128, PB, O], dt16, isOutput=False)
    nc.declare_dram_parameter("a_fb0", [128, PB, BC], dt16, isOutput=False)
    bias = nc.declare_dram_parameter("bias_col", [O, 1], _F32, isOutput=False)
    out = nc.declare_dram_parameter("out_t", [O, BC], _F32, isOutput=True)

    with tile.TileContext(nc) as tc:
        with (
            tc.tile_pool(name="fixed", bufs=1) as fixed,
            tc.tile_pool(name="ap", bufs=2) as ap_pool,
            tc.tile_pool(name="op", bufs=2) as op,
            tc.tile_pool(name="ps", bufs=1, space="PSUM") as psp,
        ):
            if variant in ("pe_only", "pe8_only", "dve_only"):
                w_fix = fixed.tile([128, PB, O], dt16, tag="wf")
                a_fix = fixed.tile([128, PB, BC], dt16, tag="af")
                r8_fix = fixed.tile([128, FB, BC], dt16, tag="r8")
                r16_fix = fixed.tile([128, PB, BC], dt16, tag="r16")
                nc.vector.memset(w_fix[:], 0.25)
                nc.vector.memset(a_fix[:], 0.25)
                nc.vector.memset(r8_fix[:], 0.25)
                nc.vector.memset(r16_fix[:], 0.25)
            if variant == "pe8_only":
                w8_fix = fixed.tile([128, PB, O], _F8, tag="wf8")
                a8_fix = fixed.tile([128, PB, BC], _F8, tag="af8")
                nc.vector.memset(w8_fix[:], 0.25)
                nc.vector.memset(a8_fix[:], 0.25)

            loop_ctx = (
                tc.For_i(
                    0,
                    loop_n,
                    1,
                    hint_engines=(
                        mybir.EngineType.PE,
                        mybir.EngineType.DVE,
                        mybir.EngineType.SP,
                        mybir.EngineType.Activation,
                    ),
                )
                if loop_n
                else contextlib.nullcontext()
            )
            with loop_ctx:
                for _rep in range(repeat):
                    if variant in ("pe_only", "pe8_only"):
                        fp8_fbs = FP8_FBS_MAP[nfp8] if variant == "pe8_only" else ()
                        psum_tiles = [
                            psp.tile([128, BC], _F32, tag=f"acc{h}", name=f"acc{h}")
                            for h in range(O // 128)
                        ]
                        if fp8_fbs:
                            psum8_tiles = [
                                psp.tile(
                                    [128, BC], _F32, tag=f"acc8{h}", name=f"acc8{h}"
                                )
                                for h in range(O // 128)
                            ]
                        last16 = max(f for f in range(FB) if f not in fp8_fbs)
                        for fb in range(FB):
                            if fb in fp8_fbs:
                                for pb in range(0, PB, 2):
                                    st8 = fb == fp8_fbs[0] and pb == 0
                                    sp8 = fb == fp8_fbs[-1] and pb == PB - 2
                                    for h in range(O // 128):
                                        nc.tensor.matmul(
                                            psum8_tiles[h][:],
                                            w8_fix[
                                                :, pb : pb + 2,
                                                h * 128 : (h + 1) * 128,
                                            ],
                                            a8_fix[:, pb : pb + 2, :],
                                            start=st8,
                                            stop=sp8,
                                            perf_mode=mybir.MatmulPerfMode.DoubleRow,
                                        )
                                continue
                            for pb in range(PB):
                                st = fb == 0 and pb == 0
                                sp = fb == last16 and pb == PB - 1
                                for h in range(O // 128):
                                    nc.tensor.matmul(
                                        psum_tiles[h][:],
                                        w_fix[:, pb, h * 128 : (h + 1) * 128],
                                        a_fix[:, pb, :],
                                        start=st,
                                        stop=sp,
                                    )
                        for h in range(O // 128):
                            out_sb = op.tile([128, BC], _F32, tag="out")
                            nc.vector.tensor_copy(out_sb[:], psum_tiles[h][:])
                            if fp8_fbs:
                                nc.vector.tensor_add(
                                    out_sb[:], out_sb[:], psum8_tiles[h][:]
                                )
                            nc.sync.dma_start(
                                out[h * 128 : (h + 1) * 128, :], out_sb[:]
                            )
                    elif variant == "dve_only":
                        for fb in range(FB):
                            a_sb = ap_pool.tile([128, PB, BC], dt16, tag="a")
                            for ch in range(2):
                                sl = slice(ch * 8, (ch + 1) * 8)
                                nc.vector.tensor_mul(
                                    a_sb[:, sl, :],
                                    r8_fix[:, fb : fb + 1, :].broadcast_to(
                                        (128, 8, BC)
                                    ),
                                    r16_fix[:, sl, :],
                                )
                        out_sb = op.tile([128, BC], _F32, tag="out")
                        nc.vector.memset(out_sb[:], 0.0)
                        nc.sync.dma_start(out[0:128, :], out_sb[:])
                    else:  # dma_only / dma_split
                        wq = nc.scalar if variant == "dma_split" else nc.sync
                        rep8_sb = ap_pool.tile([128, FB, BC], dt16, tag="rep8")
                        rep16_sb = ap_pool.tile([128, PB, BC], dt16, tag="rep16")
                        bias_sb = ap_pool.tile([128, O // 128], _F32, tag="bias")
                        nc.sync.dma_start(rep8_sb[:], rep8[:])
                        nc.sync.dma_start(rep16_sb[:], rep16[:])
                        for h in range(O // 128):
                            nc.sync.dma_start(
                                bias_sb[:, h : h + 1],
                                bias[h * 128 : (h + 1) * 128, :],
                            )
                        for fb in range(FB):
                            w_sb = ap_pool.tile([128, PB, O], dt16, tag="w")
                            wq.dma_start(w_sb[:], w_re[fb])
                        out_sb = op.tile([128, BC], _F32, tag="out")
                        nc.vector.memset(out_sb[:], 0.0)
                        nc.sync.dma_start(out[0:128, :], out_sb[:])

    nc.compile()
    _NC_CACHE[key] = nc
    return nc


def _prepare_inputs(x0, x, W, b, dtype16=np.float16, variant=None):
    """Host-side marshalling: cast to fp16, transpose, small-factor replicate,
    permute W, and shard the batch across the 8 cores."""
    if variant is None:
        variant = VARIANT
    nfp8 = 0
    f8h = False
    for o in variant.split("+")[1:]:
        if o.startswith("f8n"):
            nfp8 = int(o[3:])
        elif o == "f8h":
            f8h = True
    fp8_fbs = FP8_FBS_MAP[nfp8]
    half_fb = fp8_fbs[-1] if (f8h and nfp8) else None
    x0 = np.asarray(x0, dtype=np.float32)
    x = np.asarray(x, dtype=np.float32)
    W = np.asarray(W, dtype=np.float32)
    b = np.asarray(b, dtype=np.float32)

    x0t = np.ascontiguousarray(x0.T.astype(dtype16))  # [F, B]
    xt = np.ascontiguousarray(x.T.astype(dtype16))  # [P, B]

    # rep8[k, fb, bb] = x0t[fb*FI + k//PI, bb]
    rep8 = np.broadcast_to(
        x0t.reshape(FB, FI, B).transpose(1, 0, 2)[:, None, :, :], (FI, PI, FB, B)
    ).reshape(128, FB, B)
    # rep16[k, pb, bb] = xt[pb*PI + k%PI, bb]
    rep16 = np.broadcast_to(
        xt.reshape(PB, PI, B).transpose(1, 0, 2)[None, :, :, :], (FI, PI, PB, B)
    ).reshape(128, PB, B)

    # w_re[fb, k, pb, o] = W[o, fb*FI + k//PI, pb*PI + k%PI]
    w_re32 = (
        W.reshape(O, FB, FI, PB, PI).transpose(1, 2, 4, 3, 0).reshape(FB, 128, PB, O)
    )
    fbs16 = [fb for fb in range(FB) if fb not in fp8_fbs or fb == half_fb]
    w_re = np.ascontiguousarray(w_re32[fbs16]).astype(dtype16)
    # fp8 W is pre-scaled by 2^10: uniform(-1/128,1/128) otherwise sits in
    # e4m3's subnormal range (min normal 2^-6) with ~7x the mantissa error;
    # the kernel rescales by 2^-10 at eviction
    w8_re = (
        np.ascontiguousarray(w_re32[list(fp8_fbs)] * 1024.0).astype(_NP8)
        if nfp8
        else None
    )
    bias_col = b.astype(np.float32).reshape(O, 1)

    # fb=0's A tile, precomputed host-side with the same fp16 rounding the
    # device tensor_mul would apply (fp32 multiply, round to fp16)
    a_fb0 = (
        rep8[:, 0:1, :].astype(np.float32) * rep16.astype(np.float32)
    ).astype(dtype16)

    in_maps = []
    for c in range(NCORES):
        bs = slice(c * BC, (c + 1) * BC)
        m = {
            "rep8_x0t": np.ascontiguousarray(rep8[:, :, bs]),
            "rep16_xt": np.ascontiguousarray(rep16[:, :, bs]),
            "w_re": w_re,
            "a_fb0": np.ascontiguousarray(a_fb0[:, :, bs]),
            "bias_col": bias_col,
        }
        if nfp8:
            m["w8_re"] = w8_re
        in_maps.append(m)
    return in_maps


def kernel(x0, x, W, b, _run_kwargs=None):
    nc = _build_nc()
    in_maps = _prepare_inputs(x0, x, W, b)
    # spot-check row: rare device flakes return the (zero-initialized)
    # output buffers untouched; verify one batch row on the host and retry
    # once if the result is garbage
    ref_row = np.einsum(
        "f,p,ofp->o",
        np.asarray(x0[0], np.float32),
        np.asarray(x[0], np.float32),
        np.asarray(W, np.float32),
    ) + np.asarray(b, np.float32)
    row_scale = np.abs(ref_row).max() + 1e-30
    for _attempt in range(2):
        res = run_bass_kernel_spmd(
            nc, in_maps, core_ids=list(range(NCORES)), **(_run_kwargs or {})
        )
        # per-core results are out.T shards [O, BC]; assemble+transpose back
        out_t = np.concatenate(
            [res.results[c]["out_t"] for c in range(NCORES)], axis=1
        )  # [O, B]
        if np.abs(out_t[:, 0] - ref_row).max() / row_scale < 0.2:
            break
    if _run_kwargs:
        kernel._last_results = res
    return np.ascontiguousarray(out_t.T).reshape(B, O, 1).astype(np.float32)

